# revision 1
# baseline (speedup 1.0000x reference)
"""Multi-head self-attention (RoPE) Trainium2 Bass kernel.

Shards batch (B=8) across 8 NeuronCores, one batch element per core.

Design notes (driven by measured backend behavior):
- Cost is dominated by instruction issue, not FLOPs: K<128 or nonzero
  base-partition matmuls and strided/multi-dim-AP vector ops are several
  times more expensive than full-K contiguous ones.
- Every matmul is therefore a K=128, offset-0 contraction: each head's
  RoPE'd q/k lives in its own 128-row block of qkTp with the unused
  64-row half kept zero (scores over the zero half contribute nothing).
- Vector/scalar ops are few, wide, and contiguous: RoPE is 4 swizzle
  copies + 2 wide muls + 2 contiguous adds; V-augmentation is one
  contiguous copy per key block; softmax denominators come free as
  ones-columns in the augmented V (block [v | ones], read back as a
  [128, 2, 64] stationary AP).
- Scores cycle PSUM in the fewest ACT-gated recycles possible: 3 tiles
  per head (6+6+4 banks), with the AV accumulator in the other 2 banks.
"""
import os
import sys

# The kernel needs the 8 axon-tunneled NeuronCores visible to jax; a
# JAX_PLATFORMS=cpu pin (used by some harnesses for the reference) would
# hide them. Clear it before jax initializes through the concourse imports.
os.environ.pop("JAX_PLATFORMS", None)

sys.path.insert(0, "/opt/trn_rl_repo")

_REPS = int(os.environ.get("KREPS", "1"))
_PH = int(os.environ.get("KPHASES", "4"))  # 1=QKV 2=+scores 3=+AV 4=full
_NOEXP = int(os.environ.get("KNOEXP", "0"))  # 1: skip exp ACTs (cost attribution only)

import numpy as np
from contextlib import ExitStack

import concourse.bass as bass
import concourse.tile as tile
from concourse import bacc, mybir

f32 = mybir.dt.float32
f16 = mybir.dt.float16
AF = mybir.ActivationFunctionType
ALU = mybir.AluOpType

B, L, DIM = 8, 1024, 512
NH, HD = 8, 64
SCALE = HD ** -0.5
NCORES = 8


def _blockQ(h):
    return h // 2 if h % 2 == 0 else 8 + h // 2


def _blockK(h):
    return 4 + h // 2 if h % 2 == 0 else 12 + h // 2


def _build_nc():
    nc = bacc.Bacc("TRN2", target_bir_lowering=False, debug=False, enable_asserts=False)

    xT = nc.dram_tensor("xT", (DIM, L), f16, kind="ExternalInput")
    wq = nc.dram_tensor("wq", (DIM, 2 * DIM), f16, kind="ExternalInput")  # Q|K cols
    wv = nc.dram_tensor("wv", (DIM, DIM), f16, kind="ExternalInput")      # V cols
    wp = nc.dram_tensor("wp", (DIM, DIM), f16, kind="ExternalInput")
    cosT = nc.dram_tensor("cosT", (128, 8 * L), f16, kind="ExternalInput")
    sinT = nc.dram_tensor("sinT", (128, 8 * L), f16, kind="ExternalInput")
    bias = nc.dram_tensor("bias", (128, 8), f32, kind="ExternalInput")
    y = nc.dram_tensor("y", (L, DIM), f32, kind="ExternalOutput")

    with ExitStack() as ctx:
        tc = ctx.enter_context(tile.TileContext(nc))
        cst = ctx.enter_context(tc.tile_pool(name="cst", bufs=1))
        sc = ctx.enter_context(tc.tile_pool(name="sc", bufs=1))
        pTp = ctx.enter_context(tc.tile_pool(name="pTp", bufs=4))
        ysb = ctx.enter_context(tc.tile_pool(name="ysb", bufs=1))

        # ---- load inputs (once) ----
        xT_all = cst.tile([128, 4 * L], f16, name="t", tag="xTall")
        wq_all = cst.tile([128, 4 * 2 * DIM], f16, name="t", tag="wqall")
        wv_all = cst.tile([128, 4 * DIM], f16, name="t", tag="wvall")
        wp_all = cst.tile([128, 4 * DIM], f16, name="t", tag="wpall")
        for big, dram, w in ((xT_all, xT, L), (wq_all, wq, 2 * DIM),
                             (wv_all, wv, DIM), (wp_all, wp, DIM)):
            nc.sync.dma_start(
                big[:].rearrange("p (kc w) -> p kc w", kc=4),
                dram[:].rearrange("(kc p) w -> p kc w", p=128))
        xT_sb = [xT_all[:, i * L:(i + 1) * L] for i in range(4)]
        wq_sb = [wq_all[:, i * 2 * DIM:(i + 1) * 2 * DIM] for i in range(4)]
        wv_sb = [wv_all[:, i * DIM:(i + 1) * DIM] for i in range(4)]
        wp_sb = [wp_all[:, i * DIM:(i + 1) * DIM] for i in range(4)]
        cos_sb = cst.tile([128, 8 * L], f16, name="t", tag="cos")
        sin_sb = cst.tile([128, 8 * L], f16, name="t", tag="sin")
        bias_sb = cst.tile([128, 8], f32, name="t", tag="bias")
        nc.sync.dma_start(cos_sb[:], cosT[:])
        nc.sync.dma_start(sin_sb[:], sinT[:])
        nc.sync.dma_start(bias_sb[:], bias[:])

        # persistent working tiles
        qraw = cst.tile([128, 8 * L], f16, name="t", tag="qraw")
        qsw = cst.tile([128, 8 * L], f16, name="t", tag="qsw")
        # qkTp: 16 blocks of [128, L], one head's RoPE'd q/k per block.
        # Blocks 0..8 hold even heads (data rows 0:64), 8..16 odd heads
        # (rows 64:128); the other half of each block stays zero so all
        # scores matmuls contract over the full 128 partitions.
        qkTp = cst.tile([128, 16 * L], f16, name="t", tag="qkTp")
        # vaug[kc]: [128 keys, NH*128]; head block h: even h -> v in cols
        # 0:64 and ones in 64:128, odd h -> ones in 0:64 and v in 64:128, so
        # each head's attention numerator lands on the partition rows its
        # slot in the output layout needs (matmul stationary APs must be 2D).
        vaug = [cst.tile([128, NH * 128], f16, name="t", tag=f"vaug{i}") for i in range(8)]
        outT = [cst.tile([128, L], f16, name="t", tag=f"outT{c}") for c in range(4)]

        yall0 = ysb.tile([128, 8 * DIM], f32, name="t", tag="yall")
        nc.vector.memset(yall0[:], 0.0)
        nc.vector.memset(qkTp[:], 0.0)
        for lb in range(8):
            v3 = vaug[lb][:].rearrange("p (h2 c) -> p h2 c", h2=4)
            nc.vector.memset(v3[:, :, 64:128], 1.0)   # even-head ones
            nc.vector.memset(v3[:, :, 128:192], 1.0)  # odd-head ones

        def emit_body(rep):
            # ---------- phase 1: QK projection (2-wave ping-pong drains) ----------
            with tc.tile_pool(name=f"qkps{rep}", bufs=2, space="PSUM") as qk_ps:
                for w in range(4):
                    ps = qk_ps.tile([128, 2 * L], f32, name="t", tag="qkps")
                    for mi in range(2):
                        m = 2 * w + mi
                        for kc in range(4):
                            for qb in range(2):
                                nc.tensor.matmul(
                                    ps[:, mi * L + qb * 512:mi * L + (qb + 1) * 512],
                                    wq_sb[kc][:, m * 128:(m + 1) * 128],
                                    xT_sb[kc][:, qb * 512:(qb + 1) * 512],
                                    start=(kc == 0), stop=(kc == 3))
                    nc.scalar.copy(qraw[:, 2 * w * L:(2 * w + 2) * L], ps[:])

                # ---------- wide RoPE, all ops contiguous ----------
                for (do, so) in ((0, 32), (32, 0), (64, 96), (96, 64)):
                    nc.vector.tensor_copy(qsw[do:do + 32, :], qraw[so:so + 32, :])
                nc.vector.tensor_mul(qraw[:], qraw[:], cos_sb[:])
                nc.vector.tensor_mul(qsw[:], qsw[:], sin_sb[:])
                nc.vector.tensor_add(qkTp[0:64, 0:8 * L], qraw[0:64, :], qsw[0:64, :])
                nc.vector.tensor_add(qkTp[64:128, 8 * L:16 * L],
                                     qraw[64:128, :], qsw[64:128, :])

            # ---------- phase 1b: V projection (2-wave ping-pong drains) ----------
            with tc.tile_pool(name=f"vps{rep}", bufs=2, space="PSUM") as v_ps:
                for w in range(2):
                    vps = v_ps.tile([128, 4 * DIM], f32, name="t", tag="vps")
                    for li in range(4):
                        lb = 4 * w + li
                        for kc in range(4):
                            nc.tensor.matmul(
                                vps[:, li * DIM:(li + 1) * DIM],
                                xT_sb[kc][:, lb * 128:(lb + 1) * 128],
                                wv_sb[kc][:],
                                start=(kc == 0), stop=(kc == 3))
                    for li in range(4):
                        lb = 4 * w + li
                        v3 = vaug[lb][:].rearrange("p (h2 c) -> p h2 c", h2=4)
                        p3 = vps[:, li * DIM:(li + 1) * DIM].rearrange(
                            "p (h2 c) -> p h2 c", h2=4)
                        nc.scalar.copy(v3[:, :, 0:64], p3[:, :, 0:64])
                        nc.scalar.copy(v3[:, :, 192:256], p3[:, :, 64:128])

            # ---------- phase 2: attention ----------
            with tc.tile_pool(name=f"sps{rep}", bufs=1, space="PSUM") as s_ps, \
                 tc.tile_pool(name=f"avps{rep}", bufs=1, space="PSUM") as av_ps:
                # 3 big score tiles per head (6 banks each in turn) minimize
                # exp-ACT instruction count; X takes the remaining bank pair.
                groups = ((0, 3), (3, 3), (6, 2))
                for h in range(NH if _PH >= 2 else 0):
                    qcol = _blockQ(h) * L
                    kcol = _blockK(h) * L
                    pts = []
                    for (kb0, nkb) in groups:
                        s = s_ps.tile([128, 3072], f32, name="t", tag="s")
                        for i in range(nkb):
                            kb = kb0 + i
                            for qb in range(2):
                                nc.tensor.matmul(
                                    s[:, i * 1024 + qb * 512:i * 1024 + (qb + 1) * 512],
                                    qkTp[:, kcol + kb * 128:kcol + (kb + 1) * 128],
                                    qkTp[:, qcol + qb * 512:qcol + (qb + 1) * 512],
                                    start=True, stop=True)
                        pt = pTp.tile([128, 3072], f16, name="t", tag="pT")
                        if not _NOEXP:
                            nc.scalar.activation(pt[:, :nkb * 1024], s[:, :nkb * 1024],
                                                 AF.Exp,
                                                 bias=bias_sb[:, kb0:kb0 + 1],
                                                 scale=SCALE)
                        pts.append(pt)

                    if _PH < 3:
                        continue
                    X = av_ps.tile([128, L], f32, name="t", tag="avX")
                    for kc in range(8):
                        gi, off = (kc // 3, kc % 3) if kc < 6 else (2, kc - 6)
                        pt = pts[gi]
                        va = vaug[kc][:, h * 128:(h + 1) * 128]
                        for qb in range(2):
                            nc.tensor.matmul(
                                X[:, qb * 512:(qb + 1) * 512], va,
                                pt[:, off * 1024 + qb * 512:off * 1024 + (qb + 1) * 512],
                                start=(kc == 0), stop=(kc == 7))
                    # numerator rows match the output slot per head parity;
                    # recip needs an SBUF-staged input (custom-DVE op
                    # misreads PSUM), hence the D copy.
                    c = h // 2
                    D = sc.tile([64, L], f32, name="t", tag="D")
                    R = sc.tile([64, L], f32, name="t", tag="R")
                    if h % 2 == 0:
                        nc.vector.tensor_copy(D[:], X[64:128, :])
                        nc.vector.reciprocal_approx_fast(R[:], D[:])
                        nc.vector.tensor_mul(outT[c][0:64, :], X[0:64, :], R[:])
                    else:
                        nc.vector.tensor_copy(D[:], X[0:64, :])
                        nc.vector.reciprocal_approx_fast(R[:], D[:])
                        nc.vector.tensor_mul(outT[c][64:128, :], X[64:128, :], R[:])

            # ---------- phase 3: output projection ----------
            with tc.tile_pool(name=f"yps{rep}", bufs=2, space="PSUM") as y_ps:
                yall = yall0
                for w in range(2 if _PH >= 4 else 0):
                    yp = y_ps.tile([128, 4 * DIM], f32, name="t", tag="yps")
                    for li in range(4):
                        lb = 4 * w + li
                        for c in range(4):
                            nc.tensor.matmul(
                                yp[:, li * DIM:(li + 1) * DIM],
                                outT[c][:, lb * 128:(lb + 1) * 128],
                                wp_sb[c][:],
                                start=(c == 0), stop=(c == 3))
                    nc.scalar.copy(yall[:, 4 * w * DIM:(4 * w + 4) * DIM], yp[:])
                if _PH >= 4:
                    nc.sync.dma_start(
                        y[:].rearrange("(lb p) d -> p lb d", p=128),
                        yall[:].rearrange("p (lb d) -> p lb d", lb=8))

        for rep in range(_REPS):
            emit_body(rep)

    nc.compile()
    return nc


def _rope_tables():
    inv_freq = 1.0 / (10000.0 ** (np.arange(0, HD, 2, dtype=np.float32) / HD))
    t = np.arange(L, dtype=np.float32)
    freqs = np.outer(t, inv_freq)                      # (L, 32)
    emb = np.concatenate([freqs, freqs], axis=-1)      # (L, 64)
    cos = np.cos(emb).T                                # (64, L)
    sin = np.sin(emb).T                                # (64, L)
    sign = np.where(np.arange(HD) < HD // 2, -1.0, 1.0)[:, None].astype(np.float32)
    sin_s = sin * sign
    cosT = np.tile(cos, (2, 1)).astype(np.float16)     # (128, L)
    sinT = np.tile(sin_s, (2, 1)).astype(np.float16)   # (128, L)
    # wide tables: the same [128, L] block tiled across all 8 m-blocks
    return np.tile(cosT, (1, 8)), np.tile(sinT, (1, 8))


_NC = None


def _get_nc():
    global _NC
    if _NC is None:
        _NC = _build_nc()
    return _NC


def kernel(x, mask, w_qkv, w_proj):
    x = np.asarray(x, dtype=np.float32)
    mask = np.asarray(mask)
    w_qkv = np.asarray(w_qkv, dtype=np.float32)
    w_proj = np.asarray(w_proj, dtype=np.float32)

    nc = _get_nc()
    cosT, sinT = _rope_tables()

    wq = np.ascontiguousarray(w_qkv[:, :2 * DIM]).astype(np.float16)
    wv = np.ascontiguousarray(w_qkv[:, 2 * DIM:]).astype(np.float16)
    wp = w_proj.astype(np.float16)

    in_maps = []
    for b in range(NCORES):
        xTb = np.ascontiguousarray(x[b].T).astype(np.float16)      # (512, 1024)
        bias_b = np.where(mask[b].reshape(8, 128).T, 0.0, -1e9).astype(np.float32)
        in_maps.append({
            "xT": xTb, "wq": wq, "wv": wv, "wp": wp,
            "cosT": cosT, "sinT": sinT, "bias": bias_b,
        })

    from concourse.bass_utils import run_bass_kernel_spmd
    res = run_bass_kernel_spmd(nc, in_maps, core_ids=list(range(NCORES)))
    out = np.stack([res.results[c]["y"] for c in range(NCORES)], axis=0)
    return out.astype(np.float32)



# revision 2
# speedup vs baseline: 41.9009x; 41.9009x over previous
"""Multi-head self-attention (RoPE) Trainium2 Bass kernel.

Shards batch (B=8) across 8 NeuronCores, one batch element per core.

Design notes (driven by measured backend behavior):
- Cost is dominated by instruction issue, not FLOPs: K<128 or nonzero
  base-partition matmuls and strided/multi-dim-AP vector ops are several
  times more expensive than full-K contiguous ones.
- Every matmul is therefore a K=128, offset-0 contraction: each head's
  RoPE'd q/k lives in its own 128-row block of qkTp with the unused
  64-row half kept zero (scores over the zero half contribute nothing).
- Vector/scalar ops are few, wide, and contiguous: RoPE is 4 swizzle
  copies + 2 wide muls + 2 contiguous adds; V-augmentation is one
  contiguous copy per key block; softmax denominators come free as
  ones-columns in the augmented V (block [v | ones], read back as a
  [128, 2, 64] stationary AP).
- Scores cycle PSUM in the fewest ACT-gated recycles possible: 3 tiles
  per head (6+6+4 banks), with the AV accumulator in the other 2 banks.
"""
import os
import sys

# The kernel needs the 8 axon-tunneled NeuronCores visible to jax; a
# JAX_PLATFORMS=cpu pin (used by some harnesses for the reference) would
# hide them. Clear it before jax initializes through the concourse imports.
os.environ.pop("JAX_PLATFORMS", None)

sys.path.insert(0, "/opt/trn_rl_repo")

_REPS = int(os.environ.get("KREPS", "1"))
_PH = int(os.environ.get("KPHASES", "4"))  # 1=QKV 2=+scores 3=+AV 4=full
_NOEXP = int(os.environ.get("KNOEXP", "0"))  # 1: skip exp ACTs (cost attribution only)

import numpy as np
from contextlib import ExitStack

import concourse.bass as bass
import concourse.tile as tile
from concourse import bacc, mybir

f32 = mybir.dt.float32
f16 = mybir.dt.float16
AF = mybir.ActivationFunctionType
ALU = mybir.AluOpType

B, L, DIM = 8, 1024, 512
NH, HD = 8, 64
SCALE = HD ** -0.5
NCORES = 8


def _blockQ(h):
    return h // 2 if h % 2 == 0 else 8 + h // 2


def _blockK(h):
    return 4 + h // 2 if h % 2 == 0 else 12 + h // 2


def _build_nc():
    nc = bacc.Bacc("TRN2", target_bir_lowering=False, debug=False, enable_asserts=False)

    xT = nc.dram_tensor("xT", (DIM, L), f16, kind="ExternalInput")
    wq = nc.dram_tensor("wq", (DIM, 2 * DIM), f16, kind="ExternalInput")  # Q|K cols
    wv = nc.dram_tensor("wv", (DIM, DIM), f16, kind="ExternalInput")      # V cols
    wp = nc.dram_tensor("wp", (DIM, DIM), f16, kind="ExternalInput")
    cosT = nc.dram_tensor("cosT", (128, 8 * L), f16, kind="ExternalInput")
    sinT = nc.dram_tensor("sinT", (128, 8 * L), f16, kind="ExternalInput")
    bias = nc.dram_tensor("bias", (128, 8), f32, kind="ExternalInput")
    y = nc.dram_tensor("y", (L, DIM), f32, kind="ExternalOutput")

    with ExitStack() as ctx:
        tc = ctx.enter_context(tile.TileContext(nc))
        cst = ctx.enter_context(tc.tile_pool(name="cst", bufs=1))
        sc = ctx.enter_context(tc.tile_pool(name="sc", bufs=1))
        pTp = ctx.enter_context(tc.tile_pool(name="pTp", bufs=4))
        ysb = ctx.enter_context(tc.tile_pool(name="ysb", bufs=1))

        # ---- load inputs (once) ----
        xT_all = cst.tile([128, 4 * L], f16, name="t", tag="xTall")
        wq_all = cst.tile([128, 4 * 2 * DIM], f16, name="t", tag="wqall")
        wv_all = cst.tile([128, 4 * DIM], f16, name="t", tag="wvall")
        wp_all = cst.tile([128, 4 * DIM], f16, name="t", tag="wpall")
        for big, dram, w in ((xT_all, xT, L), (wq_all, wq, 2 * DIM),
                             (wv_all, wv, DIM), (wp_all, wp, DIM)):
            nc.sync.dma_start(
                big[:].rearrange("p (kc w) -> p kc w", kc=4),
                dram[:].rearrange("(kc p) w -> p kc w", p=128))
        xT_sb = [xT_all[:, i * L:(i + 1) * L] for i in range(4)]
        wq_sb = [wq_all[:, i * 2 * DIM:(i + 1) * 2 * DIM] for i in range(4)]
        wv_sb = [wv_all[:, i * DIM:(i + 1) * DIM] for i in range(4)]
        wp_sb = [wp_all[:, i * DIM:(i + 1) * DIM] for i in range(4)]
        cos_sb = cst.tile([128, 8 * L], f16, name="t", tag="cos")
        sin_sb = cst.tile([128, 8 * L], f16, name="t", tag="sin")
        bias_sb = cst.tile([128, 8], f32, name="t", tag="bias")
        nc.sync.dma_start(cos_sb[:], cosT[:])
        nc.sync.dma_start(sin_sb[:], sinT[:])
        nc.sync.dma_start(bias_sb[:], bias[:])

        # persistent working tiles
        qraw = cst.tile([128, 8 * L], f16, name="t", tag="qraw")
        qsw = cst.tile([128, 8 * L], f16, name="t", tag="qsw")
        # qkTp: 16 blocks of [128, L], one head's RoPE'd q/k per block.
        # Blocks 0..8 hold even heads (data rows 0:64), 8..16 odd heads
        # (rows 64:128); the other half of each block stays zero so all
        # scores matmuls contract over the full 128 partitions.
        qkTp = cst.tile([128, 16 * L], f16, name="t", tag="qkTp")
        # vaug[kc]: [128 keys, NH*128]; head block h: even h -> v in cols
        # 0:64 and ones in 64:128, odd h -> ones in 0:64 and v in 64:128, so
        # each head's attention numerator lands on the partition rows its
        # slot in the output layout needs (matmul stationary APs must be 2D).
        vaug = [cst.tile([128, NH * 128], f16, name="t", tag=f"vaug{i}") for i in range(8)]
        outT = [cst.tile([128, L], f16, name="t", tag=f"outT{c}") for c in range(4)]

        yall0 = ysb.tile([128, 8 * DIM], f32, name="t", tag="yall")
        nc.vector.memset(yall0[:], 0.0)
        nc.vector.memset(qkTp[:], 0.0)
        for lb in range(8):
            v3 = vaug[lb][:].rearrange("p (h2 c) -> p h2 c", h2=4)
            nc.vector.memset(v3[:, :, 64:128], 1.0)   # even-head ones
            nc.vector.memset(v3[:, :, 128:192], 1.0)  # odd-head ones

        def emit_body(rep):
            # ---------- phase 1: QK projection (2-wave ping-pong drains) ----------
            with tc.tile_pool(name=f"qkps{rep}", bufs=2, space="PSUM") as qk_ps:
                for w in range(4):
                    ps = qk_ps.tile([128, 2 * L], f32, name="t", tag="qkps")
                    for mi in range(2):
                        m = 2 * w + mi
                        for kc in range(4):
                            for qb in range(2):
                                nc.tensor.matmul(
                                    ps[:, mi * L + qb * 512:mi * L + (qb + 1) * 512],
                                    wq_sb[kc][:, m * 128:(m + 1) * 128],
                                    xT_sb[kc][:, qb * 512:(qb + 1) * 512],
                                    start=(kc == 0), stop=(kc == 3))
                    nc.scalar.copy(qraw[:, 2 * w * L:(2 * w + 2) * L], ps[:])

                # ---------- wide RoPE, all ops contiguous ----------
                for (do, so) in ((0, 32), (32, 0), (64, 96), (96, 64)):
                    nc.vector.tensor_copy(qsw[do:do + 32, :], qraw[so:so + 32, :])
                nc.vector.tensor_mul(qraw[:], qraw[:], cos_sb[:])
                nc.vector.tensor_mul(qsw[:], qsw[:], sin_sb[:])
                nc.vector.tensor_add(qkTp[0:64, 0:8 * L], qraw[0:64, :], qsw[0:64, :])
                nc.vector.tensor_add(qkTp[64:128, 8 * L:16 * L],
                                     qraw[64:128, :], qsw[64:128, :])

            # ---------- phase 1b: V projection (2-wave ping-pong drains) ----------
            with tc.tile_pool(name=f"vps{rep}", bufs=2, space="PSUM") as v_ps:
                for w in range(2):
                    vps = v_ps.tile([128, 4 * DIM], f32, name="t", tag="vps")
                    for li in range(4):
                        lb = 4 * w + li
                        for kc in range(4):
                            nc.tensor.matmul(
                                vps[:, li * DIM:(li + 1) * DIM],
                                xT_sb[kc][:, lb * 128:(lb + 1) * 128],
                                wv_sb[kc][:],
                                start=(kc == 0), stop=(kc == 3))
                    for li in range(4):
                        lb = 4 * w + li
                        v3 = vaug[lb][:].rearrange("p (h2 c) -> p h2 c", h2=4)
                        p3 = vps[:, li * DIM:(li + 1) * DIM].rearrange(
                            "p (h2 c) -> p h2 c", h2=4)
                        nc.scalar.copy(v3[:, :, 0:64], p3[:, :, 0:64])
                        nc.scalar.copy(v3[:, :, 192:256], p3[:, :, 64:128])

            # ---------- phase 2: attention ----------
            with tc.tile_pool(name=f"sps{rep}", bufs=1, space="PSUM") as s_ps, \
                 tc.tile_pool(name=f"avps{rep}", bufs=1, space="PSUM") as av_ps:
                # 3 big score tiles per head (6 banks each in turn) minimize
                # exp-ACT instruction count; X takes the remaining bank pair.
                groups = ((0, 3), (3, 3), (6, 2))
                for h in range(NH if _PH >= 2 else 0):
                    qcol = _blockQ(h) * L
                    kcol = _blockK(h) * L
                    pts = []
                    for (kb0, nkb) in groups:
                        s = s_ps.tile([128, 3072], f32, name="t", tag="s")
                        for i in range(nkb):
                            kb = kb0 + i
                            for qb in range(2):
                                nc.tensor.matmul(
                                    s[:, i * 1024 + qb * 512:i * 1024 + (qb + 1) * 512],
                                    qkTp[:, kcol + kb * 128:kcol + (kb + 1) * 128],
                                    qkTp[:, qcol + qb * 512:qcol + (qb + 1) * 512],
                                    start=True, stop=True)
                        pt = pTp.tile([128, 3072], f16, name="t", tag="pT")
                        if not _NOEXP:
                            nc.scalar.activation(pt[:, :nkb * 1024], s[:, :nkb * 1024],
                                                 AF.Exp,
                                                 bias=bias_sb[:, kb0:kb0 + 1],
                                                 scale=SCALE)
                        pts.append(pt)

                    if _PH < 3:
                        continue
                    X = av_ps.tile([128, L], f32, name="t", tag="avX")
                    for kc in range(8):
                        gi, off = (kc // 3, kc % 3) if kc < 6 else (2, kc - 6)
                        pt = pts[gi]
                        va = vaug[kc][:, h * 128:(h + 1) * 128]
                        for qb in range(2):
                            nc.tensor.matmul(
                                X[:, qb * 512:(qb + 1) * 512], va,
                                pt[:, off * 1024 + qb * 512:off * 1024 + (qb + 1) * 512],
                                start=(kc == 0), stop=(kc == 7))
                    # numerator rows match the output slot per head parity;
                    # recip needs an SBUF-staged input (custom-DVE op
                    # misreads PSUM), hence the D copy.
                    c = h // 2
                    D = sc.tile([64, L], f32, name="t", tag="D")
                    R = sc.tile([64, L], f32, name="t", tag="R")
                    if h % 2 == 0:
                        nc.vector.tensor_copy(D[:], X[64:128, :])
                        nc.vector.reciprocal_approx_fast(R[:], D[:])
                        nc.vector.tensor_mul(outT[c][0:64, :], X[0:64, :], R[:])
                    else:
                        nc.vector.tensor_copy(D[:], X[0:64, :])
                        nc.vector.reciprocal_approx_fast(R[:], D[:])
                        nc.vector.tensor_mul(outT[c][64:128, :], X[64:128, :], R[:])

            # ---------- phase 3: output projection ----------
            with tc.tile_pool(name=f"yps{rep}", bufs=2, space="PSUM") as y_ps:
                yall = yall0
                for w in range(2 if _PH >= 4 else 0):
                    yp = y_ps.tile([128, 4 * DIM], f32, name="t", tag="yps")
                    for li in range(4):
                        lb = 4 * w + li
                        for c in range(4):
                            nc.tensor.matmul(
                                yp[:, li * DIM:(li + 1) * DIM],
                                outT[c][:, lb * 128:(lb + 1) * 128],
                                wp_sb[c][:],
                                start=(c == 0), stop=(c == 3))
                    nc.scalar.copy(yall[:, 4 * w * DIM:(4 * w + 4) * DIM], yp[:])
                if _PH >= 4:
                    nc.sync.dma_start(
                        y[:].rearrange("(lb p) d -> p lb d", p=128),
                        yall[:].rearrange("p (lb d) -> p lb d", lb=8))

        for rep in range(_REPS):
            emit_body(rep)

    nc.compile()
    return nc


def _rope_tables():
    inv_freq = 1.0 / (10000.0 ** (np.arange(0, HD, 2, dtype=np.float32) / HD))
    t = np.arange(L, dtype=np.float32)
    freqs = np.outer(t, inv_freq)                      # (L, 32)
    emb = np.concatenate([freqs, freqs], axis=-1)      # (L, 64)
    cos = np.cos(emb).T                                # (64, L)
    sin = np.sin(emb).T                                # (64, L)
    sign = np.where(np.arange(HD) < HD // 2, -1.0, 1.0)[:, None].astype(np.float32)
    sin_s = sin * sign
    cosT = np.tile(cos, (2, 1)).astype(np.float16)     # (128, L)
    sinT = np.tile(sin_s, (2, 1)).astype(np.float16)   # (128, L)
    # wide tables: the same [128, L] block tiled across all 8 m-blocks
    return np.tile(cosT, (1, 8)), np.tile(sinT, (1, 8))


_NC = None


def _get_nc():
    global _NC
    if _NC is None:
        _NC = _build_nc()
    return _NC


def _make_in_maps(x, mask, w_qkv, w_proj):
    x = np.asarray(x, dtype=np.float32)
    mask = np.asarray(mask)
    w_qkv = np.asarray(w_qkv, dtype=np.float32)
    w_proj = np.asarray(w_proj, dtype=np.float32)

    cosT, sinT = _rope_tables()
    wq = np.ascontiguousarray(w_qkv[:, :2 * DIM]).astype(np.float16)
    wv = np.ascontiguousarray(w_qkv[:, 2 * DIM:]).astype(np.float16)
    wp = w_proj.astype(np.float16)

    in_maps = []
    for b in range(NCORES):
        xTb = np.ascontiguousarray(x[b].T).astype(np.float16)      # (512, 1024)
        bias_b = np.where(mask[b].reshape(8, 128).T, 0.0, -1e9).astype(np.float32)
        in_maps.append({
            "xT": xTb, "wq": wq, "wv": wv, "wp": wp,
            "cosT": cosT, "sinT": sinT, "bias": bias_b,
        })
    return in_maps


def kernel(x, mask, w_qkv, w_proj):
    nc = _get_nc()
    in_maps = _make_in_maps(x, mask, w_qkv, w_proj)

    from concourse.bass_utils import run_bass_kernel_spmd
    res = run_bass_kernel_spmd(nc, in_maps, core_ids=list(range(NCORES)))
    out = np.stack([res.results[c]["y"] for c in range(NCORES)], axis=0)
    return out.astype(np.float32)



# revision 14
# speedup vs baseline: 58.4270x; 1.3944x over previous
"""Multi-head self-attention (RoPE) Trainium2 Bass kernel.

Shards batch (B=8) across 8 NeuronCores, one batch element per core.

Design notes (driven by measured backend behavior):
- Cost is dominated by instruction issue, not FLOPs: K<128 or nonzero
  base-partition matmuls and strided/multi-dim-AP vector ops are several
  times more expensive than full-K contiguous ones.
- Every matmul is therefore a K=128, offset-0 contraction: each head's
  RoPE'd q/k lives in its own 128-row block of qkTp with the unused
  64-row half kept zero (scores over the zero half contribute nothing).
- Vector/scalar ops are few, wide, and contiguous: RoPE is 4 swizzle
  copies + 2 wide muls + 2 contiguous adds; V-augmentation is one
  contiguous copy per key block; softmax denominators come free as
  ones-columns in the augmented V (block [v | ones], read back as a
  [128, 2, 64] stationary AP).
- Scores cycle PSUM in the fewest ACT-gated recycles possible: 3 tiles
  per head (6+6+4 banks), with the AV accumulator in the other 2 banks.
"""
import os
import sys

# The kernel needs the 8 axon-tunneled NeuronCores visible to jax; a
# JAX_PLATFORMS=cpu pin (used by some harnesses for the reference) would
# hide them. Clear it before jax initializes through the concourse imports.
os.environ.pop("JAX_PLATFORMS", None)

sys.path.insert(0, "/opt/trn_rl_repo")

_REPS = int(os.environ.get("KREPS", "1"))
_PH = int(os.environ.get("KPHASES", "4"))  # 1=QKV 2=+scores 3=+AV 4=full
_NOEXP = int(os.environ.get("KNOEXP", "0"))  # 1: skip exp ACTs (cost attribution only)

import numpy as np
from contextlib import ExitStack

import concourse.bass as bass
import concourse.tile as tile
from concourse import bacc, mybir

f32 = mybir.dt.float32
f16 = mybir.dt.float16
AF = mybir.ActivationFunctionType
ALU = mybir.AluOpType

B, L, DIM = 8, 1024, 512
NH, HD = 8, 64
SCALE = HD ** -0.5
NCORES = 8


def _blockQ(h):
    return h // 2 if h % 2 == 0 else 8 + h // 2


def _blockK(h):
    return 4 + h // 2 if h % 2 == 0 else 12 + h // 2


def _build_nc():
    nc = bacc.Bacc("TRN2", target_bir_lowering=False, debug=False, enable_asserts=False)

    xT = nc.dram_tensor("xT", (DIM, L), f16, kind="ExternalInput")
    wq = nc.dram_tensor("wq", (DIM, 2 * DIM), f16, kind="ExternalInput")  # Q|K cols
    wv = nc.dram_tensor("wv", (DIM, DIM), f16, kind="ExternalInput")      # V cols
    wp = nc.dram_tensor("wp", (DIM, DIM), f16, kind="ExternalInput")
    cosT = nc.dram_tensor("cosT", (128, 8 * L), f16, kind="ExternalInput")
    sinT = nc.dram_tensor("sinT", (128, 8 * L), f16, kind="ExternalInput")
    bias = nc.dram_tensor("bias", (128, 8), f32, kind="ExternalInput")
    y = nc.dram_tensor("y", (L, DIM), f32, kind="ExternalOutput")

    with ExitStack() as ctx:
        tc = ctx.enter_context(tile.TileContext(nc))
        cst = ctx.enter_context(tc.tile_pool(name="cst", bufs=1))
        sc = ctx.enter_context(tc.tile_pool(name="sc", bufs=2))
        pTp = ctx.enter_context(tc.tile_pool(name="pTp", bufs=6))
        ysb = ctx.enter_context(tc.tile_pool(name="ysb", bufs=1))

        # ---- load inputs (once) ----
        xT_all = cst.tile([128, 4 * L], f16, name="t", tag="xTall")
        wq_all = cst.tile([128, 4 * 2 * DIM], f16, name="t", tag="wqall")
        wv_all = cst.tile([128, 4 * DIM], f16, name="t", tag="wvall")
        wp_all = cst.tile([128, 4 * DIM], f16, name="t", tag="wpall")
        for big, dram, w in ((xT_all, xT, L), (wq_all, wq, 2 * DIM),
                             (wv_all, wv, DIM), (wp_all, wp, DIM)):
            nc.sync.dma_start(
                big[:].rearrange("p (kc w) -> p kc w", kc=4),
                dram[:].rearrange("(kc p) w -> p kc w", p=128))
        xT_sb = [xT_all[:, i * L:(i + 1) * L] for i in range(4)]
        wq_sb = [wq_all[:, i * 2 * DIM:(i + 1) * 2 * DIM] for i in range(4)]
        wv_sb = [wv_all[:, i * DIM:(i + 1) * DIM] for i in range(4)]
        wp_sb = [wp_all[:, i * DIM:(i + 1) * DIM] for i in range(4)]
        cos_sb = cst.tile([128, 8 * L], f16, name="t", tag="cos")
        sin_sb = cst.tile([128, 8 * L], f16, name="t", tag="sin")
        bias_sb = cst.tile([128, 8], f32, name="t", tag="bias")
        nc.sync.dma_start(cos_sb[:], cosT[:])
        nc.sync.dma_start(sin_sb[:], sinT[:])
        nc.sync.dma_start(bias_sb[:], bias[:])

        # persistent working tiles
        qraw = cst.tile([128, 8 * L], f16, name="t", tag="qraw")
        qsw = cst.tile([128, 8 * L], f16, name="t", tag="qsw")
        # qkTp: 8 blocks of [128, L].  Block m<4 holds RoPE'd q of head
        # pair m (even head dims in rows 0:64, odd head in rows 64:128);
        # block 4+p holds k of pair p the same way.  Scores contract over
        # K=64 row groups via tile_position, so both halves carry data.
        qkTp = cst.tile([128, 8 * L], f16, name="t", tag="qkTp")
        # vaug[kc]: [128 keys, NH*128]; head block h: even h -> v in cols
        # 0:64 and ones in 64:128, odd h -> ones in 0:64 and v in 64:128, so
        # each head's attention numerator lands on the partition rows its
        # slot in the output layout needs (matmul stationary APs must be 2D).
        vaug = [cst.tile([128, NH * 128], f16, name="t", tag=f"vaug{i}") for i in range(8)]
        outT = [cst.tile([128, L], f16, name="t", tag=f"outT{c}") for c in range(4)]

        yall0 = ysb.tile([128, 8 * DIM], f32, name="t", tag="yall")
        nc.vector.memset(yall0[:], 0.0)
        nc.vector.memset(qkTp[:], 0.0)
        for lb in range(8):
            v3 = vaug[lb][:].rearrange("p (h2 c) -> p h2 c", h2=4)
            nc.vector.memset(v3[:, :, 64:128], 1.0)   # even-head ones
            nc.vector.memset(v3[:, :, 128:192], 1.0)  # odd-head ones

        def emit_body(rep):
            # All PSUM pools hold at most 4 banks (2-bank tiles, bufs=2) so
            # adjacent phases can coexist in PSUM and engine pipelines never
            # drain at phase boundaries.
            # ---------- phase 1: QK projection, per-m-block RoPE pipeline ----
            # Drains go to ACT (idle during phase 1); the RoPE swizzle is a
            # per-block SBUF->SBUF DMA; muls/adds per block on DVE so the
            # chain overlaps the next block's matmuls and phase 2 can start
            # as soon as the early head-pair blocks are done.
            with tc.tile_pool(name=f"qkps{rep}", bufs=2, space="PSUM") as qk_ps:
                for m in range(8):
                    ps = qk_ps.tile([128, L], f32, name="t", tag="qkps")
                    for kc in range(4):
                        for qb in range(2):
                            nc.tensor.matmul(
                                ps[:, qb * 512:(qb + 1) * 512],
                                wq_sb[kc][:, m * 128:(m + 1) * 128],
                                xT_sb[kc][:, qb * 512:(qb + 1) * 512],
                                start=(kc == 0), stop=(kc == 3))
                    mc = slice(m * L, (m + 1) * L)
                    nc.scalar.copy(qraw[:, mc], ps[:])
                    for (do, so) in ((0, 32), (32, 0), (64, 96), (96, 64)):
                        nc.sync.dma_start(qsw[do:do + 32, mc], qraw[so:so + 32, mc])
                    nc.vector.tensor_mul(qraw[:, mc], qraw[:, mc], cos_sb[:, mc])
                    nc.vector.tensor_mul(qsw[:, mc], qsw[:, mc], sin_sb[:, mc])
                    nc.vector.tensor_add(qkTp[0:64, mc],
                                         qraw[0:64, mc], qsw[0:64, mc])
                    nc.vector.tensor_add(qkTp[64:128, (8 + m) * L:(9 + m) * L],
                                         qraw[64:128, mc], qsw[64:128, mc])

            # ---------- phase 1b: V projection ----------
            with tc.tile_pool(name=f"vps{rep}", bufs=2, space="PSUM") as v_ps:
                for w in range(4):
                    vps = v_ps.tile([128, 2 * DIM], f32, name="t", tag="vps")
                    for li in range(2):
                        lb = 2 * w + li
                        for kc in range(4):
                            nc.tensor.matmul(
                                vps[:, li * DIM:(li + 1) * DIM],
                                xT_sb[kc][:, lb * 128:(lb + 1) * 128],
                                wv_sb[kc][:],
                                start=(kc == 0), stop=(kc == 3))
                    for li in range(2):
                        lb = 2 * w + li
                        v3 = vaug[lb][:].rearrange("p (h2 c) -> p h2 c", h2=4)
                        p3 = vps[:, li * DIM:(li + 1) * DIM].rearrange(
                            "p (h2 c) -> p h2 c", h2=4)
                        nc.scalar.copy(v3[:, :, 0:64], p3[:, :, 0:64])
                        nc.scalar.copy(v3[:, :, 192:256], p3[:, :, 64:128])

            # ---------- phase 2: attention ----------
            # 1-kb score groups in 2-bank f32 tiles, double-buffered: PE's
            # scores for group g+1 run during ACT's exp of group g.  AV for
            # group g-1 is emitted after scores g so PE stays busy through
            # the exp pipeline.  X double-buffered so the next head's AV
            # overlaps this head's softmax divide on DVE.
            with tc.tile_pool(name=f"sps{rep}", bufs=2, space="PSUM") as s_ps, \
                 tc.tile_pool(name=f"avps{rep}", bufs=2, space="PSUM") as av_ps:
                for h in range(NH if _PH >= 2 else 0):
                    qcol = _blockQ(h) * L
                    kcol = _blockK(h) * L
                    X = av_ps.tile([128, L], f32, name="t", tag="avX")

                    def emit_scores(kb):
                        s = s_ps.tile([128, L], f32, name="t", tag="s")
                        for qb in range(2):
                            nc.tensor.matmul(
                                s[:, qb * 512:(qb + 1) * 512],
                                qkTp[:, kcol + kb * 128:kcol + (kb + 1) * 128],
                                qkTp[:, qcol + qb * 512:qcol + (qb + 1) * 512],
                                start=True, stop=True)
                        pt = pTp.tile([128, L], f16, name="t", tag="pT")
                        if not _NOEXP:
                            nc.scalar.activation(pt[:], s[:], AF.Exp,
                                                 bias=bias_sb[:, kb:kb + 1],
                                                 scale=SCALE)
                        return pt

                    def emit_av(kc, pt):
                        if _PH < 3:
                            return
                        va = vaug[kc][:, h * 128:(h + 1) * 128]
                        for qb in range(2):
                            nc.tensor.matmul(
                                X[:, qb * 512:(qb + 1) * 512], va,
                                pt[:, qb * 512:(qb + 1) * 512],
                                start=(kc == 0), stop=(kc == 7))

                    prev = emit_scores(0)
                    for kb in range(1, 8):
                        cur = emit_scores(kb)
                        emit_av(kb - 1, prev)
                        prev = cur
                    emit_av(7, prev)

                    if _PH < 3:
                        continue
                    # numerator rows match the output slot per head parity;
                    # recip needs an SBUF-staged input (custom-DVE op
                    # misreads PSUM), hence the D copy.
                    c = h // 2
                    D = sc.tile([64, L], f32, name="t", tag="D")
                    R = sc.tile([64, L], f32, name="t", tag="R")
                    if h % 2 == 0:
                        nc.vector.tensor_copy(D[:], X[64:128, :])
                        nc.vector.reciprocal_approx_fast(R[:], D[:])
                        nc.vector.tensor_mul(outT[c][0:64, :], X[0:64, :], R[:])
                    else:
                        nc.vector.tensor_copy(D[:], X[0:64, :])
                        nc.vector.reciprocal_approx_fast(R[:], D[:])
                        nc.vector.tensor_mul(outT[c][64:128, :], X[64:128, :], R[:])

            # ---------- phase 3: output projection ----------
            with tc.tile_pool(name=f"yps{rep}", bufs=2, space="PSUM") as y_ps:
                yall = yall0
                for w in range(4 if _PH >= 4 else 0):
                    yp = y_ps.tile([128, 2 * DIM], f32, name="t", tag="yps")
                    for li in range(2):
                        lb = 2 * w + li
                        for c in range(4):
                            nc.tensor.matmul(
                                yp[:, li * DIM:(li + 1) * DIM],
                                outT[c][:, lb * 128:(lb + 1) * 128],
                                wp_sb[c][:],
                                start=(c == 0), stop=(c == 3))
                    nc.scalar.copy(yall[:, 2 * w * DIM:(2 * w + 2) * DIM], yp[:])
                if _PH >= 4:
                    nc.sync.dma_start(
                        y[:].rearrange("(lb p) d -> p lb d", p=128),
                        yall[:].rearrange("p (lb d) -> p lb d", lb=8))

        for rep in range(_REPS):
            emit_body(rep)

    nc.compile()
    return nc


def _rope_tables():
    inv_freq = 1.0 / (10000.0 ** (np.arange(0, HD, 2, dtype=np.float32) / HD))
    t = np.arange(L, dtype=np.float32)
    freqs = np.outer(t, inv_freq)                      # (L, 32)
    emb = np.concatenate([freqs, freqs], axis=-1)      # (L, 64)
    cos = np.cos(emb).T                                # (64, L)
    sin = np.sin(emb).T                                # (64, L)
    sign = np.where(np.arange(HD) < HD // 2, -1.0, 1.0)[:, None].astype(np.float32)
    sin_s = sin * sign
    cosT = np.tile(cos, (2, 1)).astype(np.float16)     # (128, L)
    sinT = np.tile(sin_s, (2, 1)).astype(np.float16)   # (128, L)
    # wide tables: the same [128, L] block tiled across all 8 m-blocks
    return np.tile(cosT, (1, 8)), np.tile(sinT, (1, 8))


_NC = None


def _get_nc():
    global _NC
    if _NC is None:
        _NC = _build_nc()
    return _NC


def _make_in_maps(x, mask, w_qkv, w_proj):
    x = np.asarray(x, dtype=np.float32)
    mask = np.asarray(mask)
    w_qkv = np.asarray(w_qkv, dtype=np.float32)
    w_proj = np.asarray(w_proj, dtype=np.float32)

    cosT, sinT = _rope_tables()
    wq = np.ascontiguousarray(w_qkv[:, :2 * DIM]).astype(np.float16)
    wv = np.ascontiguousarray(w_qkv[:, 2 * DIM:]).astype(np.float16)
    wp = w_proj.astype(np.float16)

    in_maps = []
    for b in range(NCORES):
        xTb = np.ascontiguousarray(x[b].T).astype(np.float16)      # (512, 1024)
        bias_b = np.where(mask[b].reshape(8, 128).T, 0.0, -1e9).astype(np.float32)
        in_maps.append({
            "xT": xTb, "wq": wq, "wv": wv, "wp": wp,
            "cosT": cosT, "sinT": sinT, "bias": bias_b,
        })
    return in_maps


def kernel(x, mask, w_qkv, w_proj):
    nc = _get_nc()
    in_maps = _make_in_maps(x, mask, w_qkv, w_proj)

    from concourse.bass_utils import run_bass_kernel_spmd
    res = run_bass_kernel_spmd(nc, in_maps, core_ids=list(range(NCORES)))
    out = np.stack([res.results[c]["y"] for c in range(NCORES)], axis=0)
    return out.astype(np.float32)



# revision 16
# speedup vs baseline: 582.0889x; 9.9627x over previous
"""Multi-head self-attention (RoPE) Trainium2 Bass kernel.

Shards batch (B=8) across 8 NeuronCores, one batch element per core.

Design notes (driven by measured backend behavior):
- Cost is dominated by instruction issue, not FLOPs: K<128 or nonzero
  base-partition matmuls and strided/multi-dim-AP vector ops are several
  times more expensive than full-K contiguous ones.
- Every matmul is therefore a K=128, offset-0 contraction: each head's
  RoPE'd q/k lives in its own 128-row block of qkTp with the unused
  64-row half kept zero (scores over the zero half contribute nothing).
- Vector/scalar ops are few, wide, and contiguous: RoPE is 4 swizzle
  copies + 2 wide muls + 2 contiguous adds; V-augmentation is one
  contiguous copy per key block; softmax denominators come free as
  ones-columns in the augmented V (block [v | ones], read back as a
  [128, 2, 64] stationary AP).
- Scores cycle PSUM in the fewest ACT-gated recycles possible: 3 tiles
  per head (6+6+4 banks), with the AV accumulator in the other 2 banks.
"""
import os
import sys

# The kernel needs the 8 axon-tunneled NeuronCores visible to jax; a
# JAX_PLATFORMS=cpu pin (used by some harnesses for the reference) would
# hide them. Clear it before jax initializes through the concourse imports.
os.environ.pop("JAX_PLATFORMS", None)

sys.path.insert(0, "/opt/trn_rl_repo")

_REPS = int(os.environ.get("KREPS", "1"))
_PH = int(os.environ.get("KPHASES", "4"))  # 1=QKV 2=+scores 3=+AV 4=full
_NOEXP = int(os.environ.get("KNOEXP", "0"))  # 1: skip exp ACTs (cost attribution only)

import numpy as np
from contextlib import ExitStack

import concourse.bass as bass
import concourse.tile as tile
from concourse import bacc, mybir

f32 = mybir.dt.float32
f16 = mybir.dt.float16
AF = mybir.ActivationFunctionType
ALU = mybir.AluOpType

B, L, DIM = 8, 1024, 512
NH, HD = 8, 64
SCALE = HD ** -0.5
NCORES = 8


def _blockQ(h):
    return h // 2 if h % 2 == 0 else 8 + h // 2


def _blockK(h):
    return 4 + h // 2 if h % 2 == 0 else 12 + h // 2


def _build_nc():
    nc = bacc.Bacc("TRN2", target_bir_lowering=False, debug=False, enable_asserts=False)

    xT = nc.dram_tensor("xT", (DIM, L), f16, kind="ExternalInput")
    wq = nc.dram_tensor("wq", (DIM, 2 * DIM), f16, kind="ExternalInput")  # Q|K cols
    wv = nc.dram_tensor("wv", (DIM, DIM), f16, kind="ExternalInput")      # V cols
    wp = nc.dram_tensor("wp", (DIM, DIM), f16, kind="ExternalInput")
    cosT = nc.dram_tensor("cosT", (128, 8 * L), f16, kind="ExternalInput")
    sinT = nc.dram_tensor("sinT", (128, 8 * L), f16, kind="ExternalInput")
    bias = nc.dram_tensor("bias", (128, 8), f32, kind="ExternalInput")
    y = nc.dram_tensor("y", (L, DIM), f32, kind="ExternalOutput")

    with ExitStack() as ctx:
        tc = ctx.enter_context(tile.TileContext(nc))
        cst = ctx.enter_context(tc.tile_pool(name="cst", bufs=1))
        sc = ctx.enter_context(tc.tile_pool(name="sc", bufs=2))
        pTp = ctx.enter_context(tc.tile_pool(name="pTp", bufs=6))
        ysb = ctx.enter_context(tc.tile_pool(name="ysb", bufs=1))

        # ---- load inputs (once) ----
        xT_all = cst.tile([128, 4 * L], f16, name="t", tag="xTall")
        wq_all = cst.tile([128, 4 * 2 * DIM], f16, name="t", tag="wqall")
        wv_all = cst.tile([128, 4 * DIM], f16, name="t", tag="wvall")
        wp_all = cst.tile([128, 4 * DIM], f16, name="t", tag="wpall")
        for big, dram, w in ((xT_all, xT, L), (wq_all, wq, 2 * DIM),
                             (wv_all, wv, DIM), (wp_all, wp, DIM)):
            nc.sync.dma_start(
                big[:].rearrange("p (kc w) -> p kc w", kc=4),
                dram[:].rearrange("(kc p) w -> p kc w", p=128))
        xT_sb = [xT_all[:, i * L:(i + 1) * L] for i in range(4)]
        wq_sb = [wq_all[:, i * 2 * DIM:(i + 1) * 2 * DIM] for i in range(4)]
        wv_sb = [wv_all[:, i * DIM:(i + 1) * DIM] for i in range(4)]
        wp_sb = [wp_all[:, i * DIM:(i + 1) * DIM] for i in range(4)]
        cos_sb = cst.tile([128, 8 * L], f16, name="t", tag="cos")
        sin_sb = cst.tile([128, 8 * L], f16, name="t", tag="sin")
        bias_sb = cst.tile([128, 8], f32, name="t", tag="bias")
        nc.sync.dma_start(cos_sb[:], cosT[:])
        nc.sync.dma_start(sin_sb[:], sinT[:])
        nc.sync.dma_start(bias_sb[:], bias[:])

        # persistent working tiles
        qraw = cst.tile([128, 8 * L], f16, name="t", tag="qraw")
        qsw = cst.tile([128, 8 * L], f16, name="t", tag="qsw")
        # qkTp: 8 blocks of [128, L].  Block m<4 holds RoPE'd q of head
        # pair m (even head dims in rows 0:64, odd head in rows 64:128);
        # block 4+p holds k of pair p the same way.  Scores contract over
        # K=64 row groups via tile_position, so both halves carry data.
        qkTp = cst.tile([128, 8 * L], f16, name="t", tag="qkTp")
        # vaug[kc]: [128 keys, NH*128]; head block h: even h -> v in cols
        # 0:64 and ones in 64:128, odd h -> ones in 0:64 and v in 64:128, so
        # each head's attention numerator lands on the partition rows its
        # slot in the output layout needs (matmul stationary APs must be 2D).
        vaug = [cst.tile([128, NH * 128], f16, name="t", tag=f"vaug{i}") for i in range(8)]
        outT = [cst.tile([128, L], f16, name="t", tag=f"outT{c}") for c in range(4)]

        yall0 = ysb.tile([128, 8 * DIM], f32, name="t", tag="yall")
        nc.vector.memset(yall0[:], 0.0)
        nc.vector.memset(qkTp[:], 0.0)
        for lb in range(8):
            v3 = vaug[lb][:].rearrange("p (h2 c) -> p h2 c", h2=4)
            nc.vector.memset(v3[:, :, 64:128], 1.0)   # even-head ones
            nc.vector.memset(v3[:, :, 128:192], 1.0)  # odd-head ones

        def emit_body(rep):
            # All PSUM pools hold at most 4 banks (2-bank tiles, bufs=2) so
            # adjacent phases can coexist in PSUM and engine pipelines never
            # drain at phase boundaries.
            # ---------- phase 1: QK projection, per-m-block RoPE pipeline ----
            # Drains go to ACT (idle during phase 1); the RoPE swizzle is a
            # per-block SBUF->SBUF DMA; muls/adds per block on DVE so the
            # chain overlaps the next block's matmuls and phase 2 can start
            # as soon as the early head-pair blocks are done.
            with tc.tile_pool(name=f"qkps{rep}", bufs=2, space="PSUM") as qk_ps:
                for m in range(8):
                    ps = qk_ps.tile([128, L], f32, name="t", tag="qkps")
                    for kc in range(4):
                        for qb in range(2):
                            nc.tensor.matmul(
                                ps[:, qb * 512:(qb + 1) * 512],
                                wq_sb[kc][:, m * 128:(m + 1) * 128],
                                xT_sb[kc][:, qb * 512:(qb + 1) * 512],
                                start=(kc == 0), stop=(kc == 3))
                    mc = slice(m * L, (m + 1) * L)
                    nc.scalar.copy(qraw[:, mc], ps[:])
                    for (do, so) in ((0, 32), (32, 0), (64, 96), (96, 64)):
                        nc.sync.dma_start(qsw[do:do + 32, mc], qraw[so:so + 32, mc])
                    nc.vector.tensor_mul(qraw[:, mc], qraw[:, mc], cos_sb[:, mc])
                    nc.vector.tensor_mul(qsw[:, mc], qsw[:, mc], sin_sb[:, mc])
                    nc.vector.tensor_add(qkTp[:, mc], qraw[:, mc], qsw[:, mc])

            # ---------- phase 1b: V projection ----------
            with tc.tile_pool(name=f"vps{rep}", bufs=2, space="PSUM") as v_ps:
                for w in range(4):
                    vps = v_ps.tile([128, 2 * DIM], f32, name="t", tag="vps")
                    for li in range(2):
                        lb = 2 * w + li
                        for kc in range(4):
                            nc.tensor.matmul(
                                vps[:, li * DIM:(li + 1) * DIM],
                                xT_sb[kc][:, lb * 128:(lb + 1) * 128],
                                wv_sb[kc][:],
                                start=(kc == 0), stop=(kc == 3))
                    for li in range(2):
                        lb = 2 * w + li
                        v3 = vaug[lb][:].rearrange("p (h2 c) -> p h2 c", h2=4)
                        p3 = vps[:, li * DIM:(li + 1) * DIM].rearrange(
                            "p (h2 c) -> p h2 c", h2=4)
                        nc.scalar.copy(v3[:, :, 0:64], p3[:, :, 0:64])
                        nc.scalar.copy(v3[:, :, 192:256], p3[:, :, 64:128])

            # ---------- phase 2: attention ----------
            # 1-kb score groups in 2-bank f32 tiles, double-buffered: PE's
            # scores for group g+1 run during ACT's exp of group g.  AV for
            # group g-1 is emitted after scores g so PE stays busy through
            # the exp pipeline.  X double-buffered so the next head's AV
            # overlaps this head's softmax divide on DVE.
            with tc.tile_pool(name=f"sps{rep}", bufs=2, space="PSUM") as s_ps, \
                 tc.tile_pool(name=f"avps{rep}", bufs=2, space="PSUM") as av_ps:
                for p in range(4 if _PH >= 2 else 0):
                    # head pair (2p, 2p+1): q in qkTp block p, k in block
                    # 4+p; even head rows 0:64, odd rows 64:128.  The two
                    # K=64 score matmuls target different PE row groups
                    # (tile_position) and run concurrently on hardware.
                    qcol = p * L
                    kcol = (4 + p) * L
                    Xe = av_ps.tile([128, L], f32, name="t", tag="avX")
                    Xo = av_ps.tile([128, L], f32, name="t", tag="avX")

                    def emit_scores(u):
                        kb, qb = u >> 1, u & 1
                        s = s_ps.tile([128, L], f32, name="t", tag="s")
                        nc.tensor.matmul(
                            s[:, 0:512],
                            qkTp[0:64, kcol + kb * 128:kcol + (kb + 1) * 128],
                            qkTp[0:64, qcol + qb * 512:qcol + (qb + 1) * 512],
                            start=True, stop=True, tile_position=(0, 0))
                        nc.tensor.matmul(
                            s[:, 512:1024],
                            qkTp[64:128, kcol + kb * 128:kcol + (kb + 1) * 128],
                            qkTp[64:128, qcol + qb * 512:qcol + (qb + 1) * 512],
                            start=True, stop=True, tile_position=(64, 0))
                        pt = pTp.tile([128, L], f16, name="t", tag="pT")
                        if not _NOEXP:
                            nc.scalar.activation(pt[:], s[:], AF.Exp,
                                                 bias=bias_sb[:, kb:kb + 1],
                                                 scale=SCALE)
                        return pt

                    def emit_av(u, pt):
                        if _PH < 3:
                            return
                        kb, qb = u >> 1, u & 1
                        e, o = 2 * p, 2 * p + 1
                        nc.tensor.matmul(
                            Xe[:, qb * 512:(qb + 1) * 512],
                            vaug[kb][:, e * 128:(e + 1) * 128],
                            pt[:, 0:512],
                            start=(kb == 0), stop=(kb == 7))
                        nc.tensor.matmul(
                            Xo[:, qb * 512:(qb + 1) * 512],
                            vaug[kb][:, o * 128:(o + 1) * 128],
                            pt[:, 512:1024],
                            start=(kb == 0), stop=(kb == 7))

                    prev = emit_scores(0)
                    for u in range(1, 16):
                        cur = emit_scores(u)
                        emit_av(u - 1, prev)
                        prev = cur
                    emit_av(15, prev)

                    if _PH < 3:
                        continue
                    # numerator rows match the output slot per head parity;
                    # recip needs an SBUF-staged input (custom-DVE op
                    # misreads PSUM), hence the D copy.
                    for X, par in ((Xe, 0), (Xo, 1)):
                        D = sc.tile([64, L], f32, name="t", tag="D")
                        R = sc.tile([64, L], f32, name="t", tag="R")
                        if par == 0:
                            nc.vector.tensor_copy(D[:], X[64:128, :])
                            nc.vector.reciprocal_approx_fast(R[:], D[:])
                            nc.vector.tensor_mul(outT[p][0:64, :], X[0:64, :], R[:])
                        else:
                            nc.vector.tensor_copy(D[:], X[0:64, :])
                            nc.vector.reciprocal_approx_fast(R[:], D[:])
                            nc.vector.tensor_mul(outT[p][64:128, :], X[64:128, :], R[:])

            # ---------- phase 3: output projection ----------
            with tc.tile_pool(name=f"yps{rep}", bufs=2, space="PSUM") as y_ps:
                yall = yall0
                for w in range(4 if _PH >= 4 else 0):
                    yp = y_ps.tile([128, 2 * DIM], f32, name="t", tag="yps")
                    for li in range(2):
                        lb = 2 * w + li
                        for c in range(4):
                            nc.tensor.matmul(
                                yp[:, li * DIM:(li + 1) * DIM],
                                outT[c][:, lb * 128:(lb + 1) * 128],
                                wp_sb[c][:],
                                start=(c == 0), stop=(c == 3))
                    nc.scalar.copy(yall[:, 2 * w * DIM:(2 * w + 2) * DIM], yp[:])
                if _PH >= 4:
                    nc.sync.dma_start(
                        y[:].rearrange("(lb p) d -> p lb d", p=128),
                        yall[:].rearrange("p (lb d) -> p lb d", lb=8))

        for rep in range(_REPS):
            emit_body(rep)

    nc.compile()
    return nc


def _rope_tables():
    inv_freq = 1.0 / (10000.0 ** (np.arange(0, HD, 2, dtype=np.float32) / HD))
    t = np.arange(L, dtype=np.float32)
    freqs = np.outer(t, inv_freq)                      # (L, 32)
    emb = np.concatenate([freqs, freqs], axis=-1)      # (L, 64)
    cos = np.cos(emb).T                                # (64, L)
    sin = np.sin(emb).T                                # (64, L)
    sign = np.where(np.arange(HD) < HD // 2, -1.0, 1.0)[:, None].astype(np.float32)
    sin_s = sin * sign
    cosT = np.tile(cos, (2, 1)).astype(np.float16)     # (128, L)
    sinT = np.tile(sin_s, (2, 1)).astype(np.float16)   # (128, L)
    # wide tables: the same [128, L] block tiled across all 8 m-blocks
    return np.tile(cosT, (1, 8)), np.tile(sinT, (1, 8))


_NC = None


def _get_nc():
    global _NC
    if _NC is None:
        _NC = _build_nc()
    return _NC


def _make_in_maps(x, mask, w_qkv, w_proj):
    x = np.asarray(x, dtype=np.float32)
    mask = np.asarray(mask)
    w_qkv = np.asarray(w_qkv, dtype=np.float32)
    w_proj = np.asarray(w_proj, dtype=np.float32)

    cosT, sinT = _rope_tables()
    wq = np.ascontiguousarray(w_qkv[:, :2 * DIM]).astype(np.float16)
    wv = np.ascontiguousarray(w_qkv[:, 2 * DIM:]).astype(np.float16)
    wp = w_proj.astype(np.float16)

    in_maps = []
    for b in range(NCORES):
        xTb = np.ascontiguousarray(x[b].T).astype(np.float16)      # (512, 1024)
        bias_b = np.where(mask[b].reshape(8, 128).T, 0.0, -1e9).astype(np.float32)
        in_maps.append({
            "xT": xTb, "wq": wq, "wv": wv, "wp": wp,
            "cosT": cosT, "sinT": sinT, "bias": bias_b,
        })
    return in_maps


def kernel(x, mask, w_qkv, w_proj):
    nc = _get_nc()
    in_maps = _make_in_maps(x, mask, w_qkv, w_proj)

    from concourse.bass_utils import run_bass_kernel_spmd
    res = run_bass_kernel_spmd(nc, in_maps, core_ids=list(range(NCORES)))
    out = np.stack([res.results[c]["y"] for c in range(NCORES)], axis=0)
    return out.astype(np.float32)

